# revision 21
# baseline (speedup 1.0000x reference)
"""Trainium2 Bass kernel: Conformer block (B=8, N=512, DIM=512, H=8, DH=64, FF=2048, CIN=1024, K=31).

Sharding: pure data-parallel over batch — each of the 8 NeuronCores processes one
batch item with the full weight set (no collectives).

Layout: activations are kept FEATURE-major ([feature, time] = x.T) on chip so that
chained matmuls need no transposes (weights stay in natural [din, dout] layout as
the stationary operand).  LayerNorm reductions over features become ones-vector
matmuls on the PE; per-time-step affine factors are broadcast across partitions
with a GPSIMD partition_broadcast.

Relative-position attention uses the shift-gather trick: qr = q @ rel_emb.T is
bounced through an internal DRAM scratch and read back with a strided
(stride = row+1, step -1) access pattern so that rel[j, i] = qr[i, i-j+512]
lands directly as the transposed score tile.  Scores are computed transposed
(dots_T[j, i]) so softmax runs over the partition axis: exp on ACT, the
denominator via a ones-column fused into the attn@v matmul, and the final
normalization as a broadcasted multiply.

The causal depthwise conv runs on the PE as 31 PSUM-accumulated matmuls per
128-channel block against diagonal stationary matrices; the diagonals are
(re)written with a single strided DMA per block (dst step = row+1).

Matmuls use float32r (1 cycle/row for N>=256); the FFN second matmul and the
depthwise conv run in bf16.
"""

import sys

for _p in ("/opt/trn_rl_repo", "/root/.axon_site/_ro/trn_rl_repo"):
    if _p not in sys.path:
        sys.path.insert(0, _p)

import numpy as np

B, N, DIM, H, DH, MULT, EXP, KW, MAXP = 8, 512, 512, 8, 64, 4, 2, 31, 512
INNER = H * DH
FF = DIM * MULT
CIN = DIM * EXP
EPS = 1e-5
P = 128
DT = DIM // P      # 4  feature tiles of the residual stream
FT = FF // P       # 16 ff hidden tiles
CT = CIN // P      # 8  conv channel tiles
NCORES = 8
PAD = KW - 1       # 30 causal pad


def build(split_waits=True):
    """Build the single-core Bass module (SPMD: same NEFF on all 8 cores)."""
    import concourse.bass as bass
    import concourse.mybir as mybir
    import concourse.tile as tile

    F32 = mybir.dt.float32
    F32R = mybir.dt.float32r
    BF16 = mybir.dt.bfloat16
    AF = mybir.ActivationFunctionType
    AL = mybir.AluOpType

    nc = bass.Bass()

    # ---------------- I/O ----------------
    xT_d = nc.dram_tensor("xT", [DIM, N], F32R, kind="ExternalInput")
    w1_d = nc.dram_tensor("w1", [DIM, FF], F32R, kind="ExternalInput")
    b1_d = nc.dram_tensor("b1", [P, FT], F32, kind="ExternalInput")
    w2_d = nc.dram_tensor("w2", [FF, DIM], BF16, kind="ExternalInput")
    b2_d = nc.dram_tensor("b2", [P, DT], F32, kind="ExternalInput")
    wq_d = nc.dram_tensor("wq", [DIM, INNER], F32R, kind="ExternalInput")
    bq_d = nc.dram_tensor("bq", [P, DT], F32, kind="ExternalInput")
    wk_d = nc.dram_tensor("wk", [DIM, INNER], F32R, kind="ExternalInput")
    bk_d = nc.dram_tensor("bk", [P, DT], F32, kind="ExternalInput")
    wv_d = nc.dram_tensor("wv", [DIM, INNER], F32R, kind="ExternalInput")
    bv_d = nc.dram_tensor("bvb", [P, INNER], F32R, kind="ExternalInput")
    wo_d = nc.dram_tensor("wo", [INNER, DIM], F32R, kind="ExternalInput")
    bo_d = nc.dram_tensor("bo", [P, DT], F32, kind="ExternalInput")
    relT_d = nc.dram_tensor("relT", [P, 2 * MAXP + 1], F32R, kind="ExternalInput")
    c1_d = nc.dram_tensor("c1", [DIM, 2 * CIN], F32R, kind="ExternalInput")
    c1a_d = nc.dram_tensor("c1a", [P, CT], F32, kind="ExternalInput")
    c1g_d = nc.dram_tensor("c1g", [P, CT], F32, kind="ExternalInput")
    dwd_d = nc.dram_tensor("dwd", [CT, P, KW], BF16, kind="ExternalInput")
    bns_d = nc.dram_tensor("bns", [P, CT], F32, kind="ExternalInput")
    bnt_d = nc.dram_tensor("bnt", [P, CT], F32, kind="ExternalInput")
    c2_d = nc.dram_tensor("c2", [CIN, DIM], F32R, kind="ExternalInput")
    c2b_d = nc.dram_tensor("c2b", [P, DT], F32, kind="ExternalInput")
    w3_d = nc.dram_tensor("w3", [DIM, FF], F32R, kind="ExternalInput")
    b3_d = nc.dram_tensor("b3", [P, FT], F32, kind="ExternalInput")
    w4_d = nc.dram_tensor("w4", [FF, DIM], BF16, kind="ExternalInput")
    b4_d = nc.dram_tensor("b4", [P, DT], F32, kind="ExternalInput")
    png_d = nc.dram_tensor("png", [P, DT], F32, kind="ExternalInput")
    pnb_d = nc.dram_tensor("pnb", [P, DT], F32, kind="ExternalInput")
    antid_d = nc.dram_tensor("antid", [P, P], F32R, kind="ExternalInput")
    onesf_d = nc.dram_tensor("onesf", [P, P], F32R, kind="ExternalInput")

    outT_d = nc.dram_tensor("outT", [DIM, N], F32, kind="ExternalOutput")

    QRW = 2 * MAXP + 1  # 1025 scratch row width
    qr_d = nc.dram_tensor("qr_scratch", [H, N, QRW], F32R, kind="Internal")

    def r32(ap):
        return ap.bitcast(F32R)

    with tile.TileContext(nc) as tc:
        with (
            nc.allow_low_precision(reason="fp32r/bf16 matmul feeds"),
            tc.tile_pool(name="cst", bufs=1) as cst,
            tc.tile_pool(name="sb", bufs=2) as sb,
            tc.tile_pool(name="ps", bufs=2, space="PSUM") as psp,
        ):

            # ---------------- constants ----------------
            ones_full = cst.tile([P, P], F32R, tag="ones_full")
            nc.sync.dma_start(ones_full[:, :], onesf_d[:, :])
            ident = cst.tile([P, P], F32R, tag="ident")
            nc.sync.dma_start(ident[:, :], antid_d[:, :])
            relT = cst.tile([P, QRW], F32R, tag="relT")
            nc.sync.dma_start(relT[:, :], relT_d[:, :])
            b1t = cst.tile([P, FT], F32, tag="b1t")
            nc.sync.dma_start(b1t[:, :], b1_d[:, :])
            b2t = cst.tile([P, DT], F32, tag="b2t")
            nc.sync.dma_start(b2t[:, :], b2_d[:, :])
            bqt = cst.tile([P, DT], F32, tag="bqt")
            nc.sync.dma_start(bqt[:, :], bq_d[:, :])
            bkt = cst.tile([P, DT], F32, tag="bkt")
            nc.sync.dma_start(bkt[:, :], bk_d[:, :])
            bvt = cst.tile([P, INNER], F32R, tag="bvt")
            nc.sync.dma_start(bvt[:, :], bv_d[:, :])
            bot = cst.tile([P, DT], F32, tag="bot")
            nc.sync.dma_start(bot[:, :], bo_d[:, :])
            c1at = cst.tile([P, CT], F32, tag="c1at")
            nc.sync.dma_start(c1at[:, :], c1a_d[:, :])
            c1gt = cst.tile([P, CT], F32, tag="c1gt")
            nc.sync.dma_start(c1gt[:, :], c1g_d[:, :])
            bnst = cst.tile([P, CT], F32, tag="bnst")
            nc.sync.dma_start(bnst[:, :], bns_d[:, :])
            bntt = cst.tile([P, CT], F32, tag="bntt")
            nc.sync.dma_start(bntt[:, :], bnt_d[:, :])
            c2bt = cst.tile([P, DT], F32, tag="c2bt")
            nc.sync.dma_start(c2bt[:, :], c2b_d[:, :])
            b3t = cst.tile([P, FT], F32, tag="b3t")
            nc.sync.dma_start(b3t[:, :], b3_d[:, :])
            b4t = cst.tile([P, DT], F32, tag="b4t")
            nc.sync.dma_start(b4t[:, :], b4_d[:, :])
            pngt = cst.tile([P, DT], F32, tag="pngt")
            nc.sync.dma_start(pngt[:, :], png_d[:, :])
            pnbt = cst.tile([P, DT], F32, tag="pnbt")
            nc.sync.dma_start(pnbt[:, :], pnb_d[:, :])

            # ---------------- load x (already transposed on host) ----------------
            xs = []
            for mt in range(DT):
                xt = sb.tile([P, N], F32R, tag="x", bufs=8)
                nc.sync.dma_start(xt[:, :], xT_d[mt * P:(mt + 1) * P, :])
                xs.append(xt)

            # ---------------- helpers ----------------
            def layer_norm_rc(xin):
                """Stats of LN over the partition (feature) axis.

                Returns r_b, c_b [128, 512] tiles with z = x*r_b + c_b."""
                ps_sum = psp.tile([P, N], F32, tag="s1", bufs=1)
                for kt in range(DT):
                    nc.tensor.matmul(ps_sum[:, :], ones_full[:, :], xin[kt][:, :],
                                     start=(kt == 0), stop=(kt == DT - 1))
                ps_sq = psp.tile([P, N], F32, tag="s2", bufs=1)
                for kt in range(DT):
                    xsq = sb.tile([P, N], F32R, tag="tmp", bufs=3)
                    nc.scalar.square(xsq[:, :], xin[kt][:, :])
                    nc.tensor.matmul(ps_sq[:, :], ones_full[:, :], xsq[:, :],
                                     start=(kt == 0), stop=(kt == DT - 1))
                m_b = sb.tile([P, N], F32, tag="mtile", bufs=1)
                nc.vector.tensor_scalar(out=m_b[:, :], in0=ps_sum[:, :],
                                        scalar1=1.0 / DIM, scalar2=None, op0=AL.mult)
                q_b = sb.tile([P, N], F32, tag="tmp", bufs=3)
                nc.scalar.mul(q_b[:, :], ps_sq[:, :], 1.0 / DIM)
                nm2 = sb.tile([P, N], F32, tag="tmp", bufs=3)
                nc.vector.scalar_tensor_tensor(nm2[:, :], m_b[:, :], -1.0, m_b[:, :],
                                               AL.mult, AL.mult)
                veps = sb.tile([P, N], F32, tag="tmp", bufs=3)
                nc.vector.scalar_tensor_tensor(veps[:, :], q_b[:, :], EPS, nm2[:, :],
                                               AL.add, AL.add)
                sd = sb.tile([P, N], F32, tag="tmp", bufs=3)
                nc.scalar.sqrt(sd[:, :], veps[:, :])
                r_b = sb.tile([P, N], F32, tag="r_b", bufs=2)
                nc.vector.reciprocal(r_b[:, :], sd[:, :])
                c_b = sb.tile([P, N], F32, tag="c_b", bufs=2)
                nc.vector.scalar_tensor_tensor(c_b[:, :], m_b[:, :], -1.0, r_b[:, :],
                                               AL.mult, AL.mult)
                return r_b, c_b

            def ln_apply(xin, r_b, c_b):
                zs = []
                for kt in range(DT):
                    t = sb.tile([P, N], F32, tag="lnt", bufs=2)
                    nc.vector.tensor_mul(t[:, :], xin[kt][:, :], r_b[:, :])
                    z = sb.tile([P, N], F32R, tag="z", bufs=4)
                    nc.vector.tensor_add(z[:, :], t[:, :], c_b[:, :])
                    zs.append(z)
                return zs

            def ff_block(xin, w_d, bt, w2bf_d, b2tt):
                """x + 0.5*ff(LN(x)); returns new residual tiles."""
                r_b, c_b = layer_norm_rc(xin)
                zs = ln_apply(xin, r_b, c_b)
                # h = swish(z @ w1 + b1), mt-outer with half-width weight tiles
                h1s = []
                for half in range(2):
                    wts = []
                    for kt in range(DT):
                        wt = sb.tile([P, FF // 2], F32R, tag="wbig", bufs=5)
                        nc.sync.dma_start(
                            wt[:, :], w_d[kt * P:(kt + 1) * P,
                                          half * (FF // 2):(half + 1) * (FF // 2)])
                        wts.append(wt)
                    for mh in range(FT // 2):
                        mt = half * (FT // 2) + mh
                        ph = psp.tile([P, N], F32, tag="acc", bufs=4)
                        for kt in range(DT):
                            nc.tensor.matmul(ph[:, :],
                                             r32(wts[kt][:, mh * P:(mh + 1) * P]),
                                             r32(zs[kt][:, :]),
                                             start=(kt == 0), stop=(kt == DT - 1))
                        sig = sb.tile([P, N], F32, tag="tmp", bufs=3)
                        nc.scalar.activation(sig[:, :], ph[:, :], AF.Sigmoid,
                                             bias=bt[:, mt:mt + 1], scale=1.0)
                        hs = sb.tile([P, N], BF16, tag="h1s", bufs=16)
                        nc.vector.scalar_tensor_tensor(hs[:, :], ph[:, :],
                                                       bt[:, mt:mt + 1], sig[:, :],
                                                       AL.add, AL.mult)
                        h1s.append(hs)
                # y = h @ w2 (bf16), kt-outer with 4 psum accumulators
                pys = [psp.tile([P, N], F32, tag="acc", bufs=4, name=f"pys{i}") for i in range(DT)]
                for kt in range(FT):
                    wt = sb.tile([P, DIM], BF16, tag="wsmb", bufs=6)
                    nc.sync.dma_start(wt[:, :], w2bf_d[kt * P:(kt + 1) * P, :])
                    for mt in range(DT):
                        nc.tensor.matmul(pys[mt][:, :], wt[:, mt * P:(mt + 1) * P],
                                         h1s[kt][:, :],
                                         start=(kt == 0), stop=(kt == FT - 1))
                xo = []
                for mt in range(DT):
                    t = sb.tile([P, N], F32R, tag="x", bufs=8)
                    nc.vector.scalar_tensor_tensor(t[:, :], pys[mt][:, :],
                                                   b2tt[:, mt:mt + 1], xin[mt][:, :],
                                                   AL.add, AL.add)
                    xo.append(t)
                return xo

            # ================= ff1 =================
            x1 = ff_block(xs, w1_d, b1t, w2_d, b2t)

            # ================= attention =================
            r_b, c_b = layer_norm_rc(x1)
            zs = ln_apply(x1, r_b, c_b)

            def proj_qk(w_dram, bias_t, tag):
                wts = []
                for kt in range(DT):
                    wt = sb.tile([P, INNER], F32R, tag="wsm", bufs=5)
                    nc.sync.dma_start(wt[:, :], w_dram[kt * P:(kt + 1) * P, :])
                    wts.append(wt)
                outs = []
                for mt in range(DT):
                    pq = psp.tile([P, N], F32, tag="mm", bufs=2)
                    for kt in range(DT):
                        nc.tensor.matmul(pq[:, :], r32(wts[kt][:, mt * P:(mt + 1) * P]),
                                         r32(zs[kt][:, :]),
                                         start=(kt == 0), stop=(kt == DT - 1))
                    qt = sb.tile([P, N], F32R, tag=tag, bufs=4)
                    nc.vector.tensor_scalar(out=qt[:, :], in0=pq[:, :],
                                            scalar1=bias_t[:, mt:mt + 1], scalar2=None,
                                            op0=AL.add)
                    outs.append(qt)
                return outs

            qTs = proj_qk(wq_d, bqt, "qT")
            kTs = proj_qk(wk_d, bkt, "kT")

            # v in time-major layout with a trailing ones column per head
            wvts = []
            for kt in range(DT):
                wt = sb.tile([P, INNER], F32R, tag="wsm", bufs=5)
                nc.sync.dma_start(wt[:, :], wv_d[kt * P:(kt + 1) * P, :])
                wvts.append(wt)
            vext = []
            for jt in range(DT):
                pv = psp.tile([P, N], F32, tag="mm", bufs=2)
                for kt in range(DT):
                    nc.tensor.matmul(pv[:, :], r32(zs[kt][:, jt * P:(jt + 1) * P]),
                                     r32(wvts[kt][:, :]),
                                     start=(kt == 0), stop=(kt == DT - 1))
                vx = sb.tile([P, H * 2 * DH], F32R, tag="vext", bufs=4)
                vw = vx[:, 0:H * 2 * DH].rearrange("p (h c) -> p h c", c=2 * DH)
                nc.vector.scalar_tensor_tensor(
                    vw[:, :, 0:DH],
                    pv[:, :].rearrange("p (h d) -> p h d", h=H), 1.0,
                    bvt[:, :].rearrange("p (h d) -> p h d", h=H),
                    AL.mult, AL.add)
                nc.vector.tensor_copy(
                    vw[:, :, DH:2 * DH],
                    ones_full[:, None, 0:DH].broadcast_to([P, H, DH]))
                vext.append(vx)

            # qr = q @ relT, bounced through DRAM scratch (640-wide windows)
            for h in range(H):
                hb = (h % 2) * DH
                for it in range(DT):
                    lq = qTs[h // 2][hb:hb + DH, it * P:(it + 1) * P]
                    w0 = it * P + 1
                    pq1 = psp.tile([P, 320], F32, tag="acc", bufs=4)
                    nc.tensor.matmul(pq1[:, :], r32(lq),
                                     r32(relT[hb:hb + DH, w0:w0 + 320]),
                                     start=True, stop=True)
                    pq2 = psp.tile([P, 320], F32, tag="acc", bufs=4)
                    nc.tensor.matmul(pq2[:, :], r32(lq),
                                     r32(relT[hb:hb + DH, w0 + 320:w0 + 640]),
                                     start=True, stop=True)
                    qt = sb.tile([P, 640], F32R, tag="qt", bufs=2)
                    nc.scalar.copy(qt[:, 0:320], pq1[:, :])
                    nc.vector.tensor_copy(qt[:, 320:640], pq2[:, :])
                    nc.scalar.dma_start(
                        qr_d[h, it * P:(it + 1) * P, w0:w0 + 640], qt[:, :])

            # scores (transposed), softmax over partitions, attn @ v
            oTs = [sb.tile([P, N], F32R, tag="oT", bufs=4, name=f"oTs{i}") for i in range(DT)]
            for h in range(H):
                hb = (h % 2) * DH
                exps = []
                for jt in range(DT):
                    rel = sb.tile([P, N], F32R, tag="rel", bufs=2)
                    # row-reversed gather (positive partition step for walrus);
                    # the anti-diagonal `ident` un-reverses it in the matmul
                    src = bass.AP(qr_d, h * N * QRW + 3 * P + 1 - P * jt,
                                  [[1, P], [QRW + 1, N]])
                    nc.scalar.dma_start(rel[:, :], src)
                    pss = psp.tile([P, N], F32, tag="acc", bufs=4)
                    nc.tensor.matmul(pss[:, :],
                                     r32(kTs[h // 2][hb:hb + DH, jt * P:(jt + 1) * P]),
                                     r32(qTs[h // 2][hb:hb + DH, :]),
                                     start=True, stop=False)
                    nc.tensor.matmul(pss[:, :], r32(ident[:, :]), r32(rel[:, :]),
                                     start=False, stop=True)
                    e = sb.tile([P, N], F32R, tag="exp", bufs=4)
                    nc.scalar.activation(e[:, :], pss[:, :], AF.Exp)
                    exps.append(e)
                po = psp.tile([P, N], F32, tag="mm", bufs=2)
                for jt in range(DT):
                    nc.tensor.matmul(po[:, :],
                                     r32(vext[jt][:, h * 2 * DH:(h + 1) * 2 * DH]),
                                     r32(exps[jt][:, :]),
                                     start=(jt == 0), stop=(jt == DT - 1))
                rb = sb.tile([DH, N], F32, tag="dwt", bufs=3)
                nc.vector.reciprocal(rb[:, :], po[DH:2 * DH, :])
                nc.vector.tensor_mul(oTs[h // 2][hb:hb + DH, :], po[0:DH, :],
                                     rb[:, :])

            # out-projection + residual
            wots = []
            for kt in range(DT):
                wt = sb.tile([P, DIM], F32R, tag="wsm", bufs=5)
                nc.sync.dma_start(wt[:, :], wo_d[kt * P:(kt + 1) * P, :])
                wots.append(wt)
            pas = [psp.tile([P, N], F32, tag="acc", bufs=4, name=f"pas{i}") for i in range(DT)]
            for kt in range(DT):
                for mt in range(DT):
                    nc.tensor.matmul(pas[mt][:, :], r32(wots[kt][:, mt * P:(mt + 1) * P]),
                                     r32(oTs[kt][:, :]),
                                     start=(kt == 0), stop=(kt == DT - 1))
            x2 = []
            for mt in range(DT):
                t = sb.tile([P, N], F32R, tag="x", bufs=8)
                nc.vector.scalar_tensor_tensor(t[:, :], pas[mt][:, :],
                                               bot[:, mt:mt + 1], x1[mt][:, :],
                                               AL.add, AL.add)
                x2.append(t)

            # ================= conv module =================
            glus = []
            for half in range(2):
                c1ts = []
                for kt in range(DT):
                    wt = sb.tile([P, CIN], F32R, tag="wbig", bufs=5)
                    nc.sync.dma_start(
                        wt[:, :], c1_d[kt * P:(kt + 1) * P,
                                       half * CIN:(half + 1) * CIN])
                    c1ts.append(wt)
                for ch in range(CT // 2):
                    ct = half * (CT // 2) + ch
                    pa = psp.tile([P, N], F32, tag="acc", bufs=4)
                    pg = psp.tile([P, N], F32, tag="acc", bufs=4)
                    for kt in range(DT):
                        nc.tensor.matmul(pa[:, :], r32(c1ts[kt][:, ch * P:ch * P + P]),
                                         r32(x2[kt][:, :]),
                                         start=(kt == 0), stop=(kt == DT - 1))
                    for kt in range(DT):
                        nc.tensor.matmul(pg[:, :],
                                         r32(c1ts[kt][:, (CT // 2 + ch) * P:
                                                      (CT // 2 + ch) * P + P]),
                                         r32(x2[kt][:, :]),
                                         start=(kt == 0), stop=(kt == DT - 1))
                    sig = sb.tile([P, N], F32, tag="tmp", bufs=3)
                    nc.scalar.activation(sig[:, :], pg[:, :], AF.Sigmoid,
                                         bias=c1gt[:, ct:ct + 1], scale=1.0)
                    glu = sb.tile([P, PAD + N], BF16, tag="glu", bufs=4)
                    nc.vector.memset(glu[:, 0:PAD], 0.0)
                    nc.vector.scalar_tensor_tensor(glu[:, PAD:PAD + N], pa[:, :],
                                                   c1at[:, ct:ct + 1], sig[:, :],
                                                   AL.add, AL.mult)
                    glus.append(glu)

            # wait: c1 column order is [a(0:1024) | g(1024:2048)]; with the
            # half-split above, half 0 columns 0:1024 are a(ct=0..7), half 1 is g.
            # Reorder handled on host: c1 is passed pre-interleaved per half.

            # depthwise conv as 31 diagonal matmuls per channel block
            dgs = []
            for i in range(2):
                dg = sb.tile([P, KW * P], BF16, tag=f"dg{i}", bufs=1, name=f"dg{i}")
                nc.vector.memset(dg[:, :], 0.0)
                dgs.append(dg)
            hcs = []
            for ct in range(CT):
                dg = dgs[ct % 2]
                rs = dg.tensor.shape[-1]
                nc.scalar.dma_start(bass.AP(dg.tensor, 0, [[rs + 1, P], [P, KW]]),
                                  dwd_d[ct, :, :])
                pd = psp.tile([P, N], F32, tag="mm", bufs=2)
                for k in range(KW):
                    nc.tensor.matmul(pd[:, :], dg[:, k * P:(k + 1) * P],
                                     glus[ct][:, k:k + N],
                                     start=(k == 0), stop=(k == KW - 1))
                sig = sb.tile([P, N], F32, tag="dwt", bufs=3)
                nc.scalar.activation(sig[:, :], pd[:, :], AF.Sigmoid,
                                     bias=bntt[:, ct:ct + 1], scale=bnst[:, ct:ct + 1])
                u = sb.tile([P, N], F32, tag="dwt", bufs=3)
                nc.vector.tensor_scalar(out=u[:, :], in0=pd[:, :],
                                        scalar1=bnst[:, ct:ct + 1],
                                        scalar2=bntt[:, ct:ct + 1],
                                        op0=AL.mult, op1=AL.add)
                hc = sb.tile([P, N], F32R, tag="hc", bufs=6)
                nc.vector.tensor_mul(hc[:, :], u[:, :], sig[:, :])
                hcs.append(hc)

            # conv2 + residual (kt-outer)
            pcs = [psp.tile([P, N], F32, tag="acc", bufs=4, name=f"pcs{i}") for i in range(DT)]
            for kt in range(CT):
                wt = sb.tile([P, DIM], F32R, tag="wsm", bufs=5)
                nc.sync.dma_start(wt[:, :], c2_d[kt * P:(kt + 1) * P, :])
                for mt in range(DT):
                    nc.tensor.matmul(pcs[mt][:, :], r32(wt[:, mt * P:(mt + 1) * P]),
                                     r32(hcs[kt][:, :]),
                                     start=(kt == 0), stop=(kt == CT - 1))
            x3 = []
            for mt in range(DT):
                t = sb.tile([P, N], F32R, tag="x", bufs=8)
                nc.vector.scalar_tensor_tensor(t[:, :], pcs[mt][:, :],
                                               c2bt[:, mt:mt + 1], x2[mt][:, :],
                                               AL.add, AL.add)
                x3.append(t)

            # ================= ff2 =================
            x4 = ff_block(x3, w3_d, b3t, w4_d, b4t)

            # ================= post-LN =================
            r_b, c_b = layer_norm_rc(x4)
            for mt in range(DT):
                t = sb.tile([P, N], F32, tag="lnt", bufs=2)
                nc.vector.tensor_mul(t[:, :], x4[mt][:, :], r_b[:, :])
                t2 = sb.tile([P, N], F32, tag="lnt2", bufs=1)
                nc.vector.tensor_add(t2[:, :], t[:, :], c_b[:, :])
                ot = sb.tile([P, N], F32, tag="outt", bufs=2)
                nc.vector.tensor_scalar(out=ot[:, :], in0=t2[:, :],
                                        scalar1=pngt[:, mt:mt + 1],
                                        scalar2=pnbt[:, mt:mt + 1],
                                        op0=AL.mult, op1=AL.add)
                nc.scalar.dma_start(outT_d[mt * P:(mt + 1) * P, :], ot[:, :])

    if split_waits:
        _split_matmul_waits(nc, mybir)
    return nc


def _split_matmul_waits(nc, mybir):
    """This walrus build rejects engine instructions carrying more than one
    sync wait; hoist the extras onto EventSemaphore instructions on the same
    engine queue right before the instruction."""
    fn = nc.m.functions[0]
    ctr = 0
    for blk in fn.blocks:
        out = []
        changed = False
        for ins in blk.instructions:
            si = ins.sync_info
            if (si is not None and si.on_wait and len(si.on_wait) > 1
                    and not isinstance(ins, (mybir.InstEventSemaphore,
                                             mybir.InstNoOp))):
                waits = list(si.on_wait)
                for w in waits[:-1]:
                    ev = mybir.InstNoOp(
                        name=f"EVW-{ctr}", ins=[], outs=[],
                        sync_info=mybir.SyncInfo(on_wait=[w], on_update=[]))
                    ev.engine = ins.engine
                    ctr += 1
                    out.append(ev)
                ins.sync_info = mybir.SyncInfo(
                    on_wait=[waits[-1]], on_update=list(si.on_update or []))
                changed = True
            out.append(ins)
        if changed:
            blk.instructions = out


def prep_inputs(inputs):
    """Host-side preprocessing: fold LN affines / scales / biases into weights."""
    import ml_dtypes

    f = np.float32
    bf = ml_dtypes.bfloat16
    ii = {k: np.asarray(v, dtype=f) for k, v in inputs.items()}

    def colmaj(b, nb):
        return np.ascontiguousarray(b.astype(f).reshape(nb, P).T)

    g1, be1 = ii["ff1_ln_g"], ii["ff1_ln_b"]
    w1 = np.ascontiguousarray(g1[:, None] * ii["ff1_w1"])
    b1 = colmaj(be1 @ ii["ff1_w1"] + ii["ff1_b1"], FT)
    w2 = np.ascontiguousarray((0.5 * ii["ff1_w2"]).astype(bf))
    b2 = colmaj(0.5 * ii["ff1_b2"], DT)

    ag, ab = ii["attn_ln_g"], ii["attn_ln_b"]
    sc = DH ** -0.5
    wq = np.ascontiguousarray(ag[:, None] * ii["wq"] * sc)
    bq = colmaj((ab @ ii["wq"] + ii["bq"]) * sc, DT)
    wkv, bkv = ii["wkv"], ii["bkv"]
    wk = np.ascontiguousarray(ag[:, None] * wkv[:, :INNER])
    bk = colmaj(ab @ wkv[:, :INNER] + bkv[:INNER], DT)
    wv = np.ascontiguousarray(ag[:, None] * wkv[:, INNER:])
    bv = np.ascontiguousarray(np.broadcast_to(
        ab @ wkv[:, INNER:] + bkv[INNER:], (P, INNER)))
    wo = np.ascontiguousarray(ii["wo"])
    bo = colmaj(ii["bo"], DT)
    # relT rows: head feature d lives at partition (h%2)*64 + d -> duplicate rows
    rT = ii["rel_emb"].T  # [64, 1025]
    relT = np.ascontiguousarray(np.concatenate([rT, rT], axis=0))  # [128, 1025]

    # c1 columns reordered to match the kernel's half-split loop:
    # half h covers channel blocks ct=4h..4h+3 and lays out [a-cols | g-cols]
    w = ii["conv1_w"]
    HC = CIN // 2  # 512
    c1 = np.ascontiguousarray(np.concatenate(
        [w[:, 0:HC], w[:, CIN:CIN + HC], w[:, HC:CIN], w[:, CIN + HC:]], axis=1))
    c1b = ii["conv1_b"]
    c1a = colmaj(c1b[:CIN], CT)
    c1g = colmaj(c1b[CIN:], CT)
    dwd = np.ascontiguousarray(ii["dw_w"].reshape(CT, P, KW).astype(bf))
    inv = 1.0 / np.sqrt(ii["bn_var"] + EPS)
    s = inv * ii["bn_g"]
    t = ii["bn_b"] - ii["bn_mean"] * s
    bns = colmaj(s, CT)
    bnt = colmaj(t + s * ii["dw_b"], CT)
    c2 = np.ascontiguousarray(ii["conv2_w"])
    c2b = colmaj(ii["conv2_b"], DT)

    g3, be3 = ii["ff2_ln_g"], ii["ff2_ln_b"]
    w3 = np.ascontiguousarray(g3[:, None] * ii["ff2_w1"])
    b3 = colmaj(be3 @ ii["ff2_w1"] + ii["ff2_b1"], FT)
    w4 = np.ascontiguousarray((0.5 * ii["ff2_w2"]).astype(bf))
    b4 = colmaj(0.5 * ii["ff2_b2"], DT)

    png = colmaj(ii["pn_g"], DT)
    pnb = colmaj(ii["pn_b"], DT)

    shared = dict(w1=w1, b1=b1, w2=w2, b2=b2, wq=wq, bq=bq, wk=wk, bk=bk,
                  wv=wv, bvb=bv, wo=wo, bo=bo, relT=relT, c1=c1, c1a=c1a,
                  c1g=c1g, dwd=dwd, bns=bns, bnt=bnt, c2=c2, c2b=c2b,
                  w3=w3, b3=b3, w4=w4, b4=b4, png=png, pnb=pnb,
                  antid=np.ascontiguousarray(np.eye(P, dtype=f)[::-1]),
                  onesf=np.ones((P, P), dtype=f))
    x = ii["x"]
    in_maps = []
    for b in range(NCORES):
        m = dict(shared)
        m["xT"] = np.ascontiguousarray(x[b].T)
        in_maps.append(m)
    return in_maps


_BUILT = None


def run(inputs, trace=False):
    global _BUILT
    from concourse import bass_utils

    in_maps = prep_inputs(inputs)
    if _BUILT is None:
        _BUILT = build()
    res = bass_utils.run_bass_kernel_spmd(
        _BUILT, in_maps, core_ids=list(range(NCORES)), trace=trace)
    out = np.stack([np.asarray(r["outT"]).T for r in res.results])
    return np.ascontiguousarray(out.astype(np.float32)), res


def kernel(**inputs):
    out, _ = run(inputs, trace=False)
    return out


# revision 23
# speedup vs baseline: 8.2835x; 8.2835x over previous
"""Trainium2 Bass kernel: Conformer block (B=8, N=512, DIM=512, H=8, DH=64, FF=2048, CIN=1024, K=31).

Sharding: pure data-parallel over batch — each of the 8 NeuronCores processes one
batch item with the full weight set (no collectives).

Layout: activations are kept FEATURE-major ([feature, time] = x.T) on chip so that
chained matmuls need no transposes (weights stay in natural [din, dout] layout as
the stationary operand).  LayerNorm reductions over features become ones-vector
matmuls on the PE; per-time-step affine factors are broadcast across partitions
with a GPSIMD partition_broadcast.

Relative-position attention uses the shift-gather trick: qr = q @ rel_emb.T is
bounced through an internal DRAM scratch and read back with a strided
(stride = row+1, step -1) access pattern so that rel[j, i] = qr[i, i-j+512]
lands directly as the transposed score tile.  Scores are computed transposed
(dots_T[j, i]) so softmax runs over the partition axis: exp on ACT, the
denominator via a ones-column fused into the attn@v matmul, and the final
normalization as a broadcasted multiply.

The causal depthwise conv runs on the PE as 31 PSUM-accumulated matmuls per
128-channel block against diagonal stationary matrices; the diagonals are
(re)written with a single strided DMA per block (dst step = row+1).

Matmuls use float32r (1 cycle/row for N>=256); the FFN second matmul and the
depthwise conv run in bf16.
"""

import sys

for _p in ("/opt/trn_rl_repo", "/root/.axon_site/_ro/trn_rl_repo"):
    if _p not in sys.path:
        sys.path.insert(0, _p)

import numpy as np

B, N, DIM, H, DH, MULT, EXP, KW, MAXP = 8, 512, 512, 8, 64, 4, 2, 31, 512
INNER = H * DH
FF = DIM * MULT
CIN = DIM * EXP
EPS = 1e-5
P = 128
DT = DIM // P      # 4  feature tiles of the residual stream
FT = FF // P       # 16 ff hidden tiles
CT = CIN // P      # 8  conv channel tiles
NCORES = 8
PAD = KW - 1       # 30 causal pad


def build(split_waits=True):
    """Build the single-core Bass module (SPMD: same NEFF on all 8 cores)."""
    import concourse.bass as bass
    import concourse.mybir as mybir
    import concourse.tile as tile

    F32 = mybir.dt.float32
    F32R = mybir.dt.float32r
    BF16 = mybir.dt.bfloat16
    AF = mybir.ActivationFunctionType
    AL = mybir.AluOpType

    nc = bass.Bass()

    # ---------------- I/O ----------------
    xT_d = nc.dram_tensor("xT", [DIM, N], F32R, kind="ExternalInput")
    w1_d = nc.dram_tensor("w1", [DIM, FF], F32R, kind="ExternalInput")
    b1_d = nc.dram_tensor("b1", [P, FT], F32, kind="ExternalInput")
    w2_d = nc.dram_tensor("w2", [FF, DIM], BF16, kind="ExternalInput")
    b2_d = nc.dram_tensor("b2", [P, DT], F32, kind="ExternalInput")
    wq_d = nc.dram_tensor("wq", [DIM, INNER], F32R, kind="ExternalInput")
    bq_d = nc.dram_tensor("bq", [P, DT], F32, kind="ExternalInput")
    wk_d = nc.dram_tensor("wk", [DIM, INNER], F32R, kind="ExternalInput")
    bk_d = nc.dram_tensor("bk", [P, DT], F32, kind="ExternalInput")
    wv_d = nc.dram_tensor("wv", [DIM, INNER], F32R, kind="ExternalInput")
    bv_d = nc.dram_tensor("bvb", [P, INNER], F32R, kind="ExternalInput")
    wo_d = nc.dram_tensor("wo", [INNER, DIM], F32R, kind="ExternalInput")
    bo_d = nc.dram_tensor("bo", [P, DT], F32, kind="ExternalInput")
    relT_d = nc.dram_tensor("relT", [P, 2 * MAXP + 1], F32R, kind="ExternalInput")
    c1_d = nc.dram_tensor("c1", [DIM, 2 * CIN], F32R, kind="ExternalInput")
    c1a_d = nc.dram_tensor("c1a", [P, CT], F32, kind="ExternalInput")
    c1g_d = nc.dram_tensor("c1g", [P, CT], F32, kind="ExternalInput")
    dwd_d = nc.dram_tensor("dwdiag", [CT, P, KW * P], BF16, kind="ExternalInput")
    bns_d = nc.dram_tensor("bns", [P, CT], F32, kind="ExternalInput")
    bnt_d = nc.dram_tensor("bnt", [P, CT], F32, kind="ExternalInput")
    c2_d = nc.dram_tensor("c2", [CIN, DIM], F32R, kind="ExternalInput")
    c2b_d = nc.dram_tensor("c2b", [P, DT], F32, kind="ExternalInput")
    w3_d = nc.dram_tensor("w3", [DIM, FF], F32R, kind="ExternalInput")
    b3_d = nc.dram_tensor("b3", [P, FT], F32, kind="ExternalInput")
    w4_d = nc.dram_tensor("w4", [FF, DIM], BF16, kind="ExternalInput")
    b4_d = nc.dram_tensor("b4", [P, DT], F32, kind="ExternalInput")
    png_d = nc.dram_tensor("png", [P, DT], F32, kind="ExternalInput")
    pnb_d = nc.dram_tensor("pnb", [P, DT], F32, kind="ExternalInput")
    antid_d = nc.dram_tensor("antid", [P, P], F32R, kind="ExternalInput")
    onesf_d = nc.dram_tensor("onesf", [P, P], F32R, kind="ExternalInput")

    outT_d = nc.dram_tensor("outT", [DIM, N], F32, kind="ExternalOutput")

    QRW = 2 * MAXP + 1  # 1025 scratch row width
    qr_d = nc.dram_tensor("qr_scratch", [H, N, QRW], F32R, kind="Internal")

    def r32(ap):
        return ap.bitcast(F32R)

    with tile.TileContext(nc) as tc:
        with (
            nc.allow_low_precision(reason="fp32r/bf16 matmul feeds"),
            tc.tile_pool(name="cst", bufs=1) as cst,
            tc.tile_pool(name="sb", bufs=2) as sb,
            tc.tile_pool(name="ps", bufs=2, space="PSUM") as psp,
        ):

            # ---------------- constants ----------------
            ones_full = cst.tile([P, P], F32R, tag="ones_full")
            nc.sync.dma_start(ones_full[:, :], onesf_d[:, :])
            ident = cst.tile([P, P], F32R, tag="ident")
            nc.sync.dma_start(ident[:, :], antid_d[:, :])
            relT = cst.tile([P, QRW], F32R, tag="relT")
            nc.sync.dma_start(relT[:, :], relT_d[:, :])
            b1t = cst.tile([P, FT], F32, tag="b1t")
            nc.sync.dma_start(b1t[:, :], b1_d[:, :])
            b2t = cst.tile([P, DT], F32, tag="b2t")
            nc.sync.dma_start(b2t[:, :], b2_d[:, :])
            bqt = cst.tile([P, DT], F32, tag="bqt")
            nc.sync.dma_start(bqt[:, :], bq_d[:, :])
            bkt = cst.tile([P, DT], F32, tag="bkt")
            nc.sync.dma_start(bkt[:, :], bk_d[:, :])
            bvt = cst.tile([P, INNER], F32R, tag="bvt")
            nc.sync.dma_start(bvt[:, :], bv_d[:, :])
            bot = cst.tile([P, DT], F32, tag="bot")
            nc.sync.dma_start(bot[:, :], bo_d[:, :])
            c1at = cst.tile([P, CT], F32, tag="c1at")
            nc.sync.dma_start(c1at[:, :], c1a_d[:, :])
            c1gt = cst.tile([P, CT], F32, tag="c1gt")
            nc.sync.dma_start(c1gt[:, :], c1g_d[:, :])
            bnst = cst.tile([P, CT], F32, tag="bnst")
            nc.sync.dma_start(bnst[:, :], bns_d[:, :])
            bntt = cst.tile([P, CT], F32, tag="bntt")
            nc.sync.dma_start(bntt[:, :], bnt_d[:, :])
            c2bt = cst.tile([P, DT], F32, tag="c2bt")
            nc.sync.dma_start(c2bt[:, :], c2b_d[:, :])
            b3t = cst.tile([P, FT], F32, tag="b3t")
            nc.sync.dma_start(b3t[:, :], b3_d[:, :])
            b4t = cst.tile([P, DT], F32, tag="b4t")
            nc.sync.dma_start(b4t[:, :], b4_d[:, :])
            pngt = cst.tile([P, DT], F32, tag="pngt")
            nc.sync.dma_start(pngt[:, :], png_d[:, :])
            pnbt = cst.tile([P, DT], F32, tag="pnbt")
            nc.sync.dma_start(pnbt[:, :], pnb_d[:, :])

            # ---------------- load x (already transposed on host) ----------------
            xs = []
            for mt in range(DT):
                xt = sb.tile([P, N], F32R, tag="x", bufs=7)
                nc.sync.dma_start(xt[:, :], xT_d[mt * P:(mt + 1) * P, :])
                xs.append(xt)

            # ---------------- helpers ----------------
            def layer_norm_rc(xin):
                """Stats of LN over the partition (feature) axis.

                Returns r_b, c_b [128, 512] tiles with z = x*r_b + c_b."""
                ps_sum = psp.tile([P, N], F32, tag="s1", bufs=1)
                for kt in range(DT):
                    nc.tensor.matmul(ps_sum[:, :], ones_full[:, :], xin[kt][:, :],
                                     start=(kt == 0), stop=(kt == DT - 1))
                ps_sq = psp.tile([P, N], F32, tag="s2", bufs=1)
                for kt in range(DT):
                    xsq = sb.tile([P, N], F32R, tag="tmp", bufs=3)
                    nc.scalar.square(xsq[:, :], xin[kt][:, :])
                    nc.tensor.matmul(ps_sq[:, :], ones_full[:, :], xsq[:, :],
                                     start=(kt == 0), stop=(kt == DT - 1))
                m_b = sb.tile([P, N], F32, tag="mtile", bufs=1)
                nc.vector.tensor_scalar(out=m_b[:, :], in0=ps_sum[:, :],
                                        scalar1=1.0 / DIM, scalar2=None, op0=AL.mult)
                q_b = sb.tile([P, N], F32, tag="tmp", bufs=3)
                nc.scalar.mul(q_b[:, :], ps_sq[:, :], 1.0 / DIM)
                nm2 = sb.tile([P, N], F32, tag="tmp", bufs=3)
                nc.vector.scalar_tensor_tensor(nm2[:, :], m_b[:, :], -1.0, m_b[:, :],
                                               AL.mult, AL.mult)
                veps = sb.tile([P, N], F32, tag="tmp", bufs=3)
                nc.vector.scalar_tensor_tensor(veps[:, :], q_b[:, :], EPS, nm2[:, :],
                                               AL.add, AL.add)
                sd = sb.tile([P, N], F32, tag="tmp", bufs=3)
                nc.scalar.sqrt(sd[:, :], veps[:, :])
                r_b = sb.tile([P, N], F32, tag="r_b", bufs=2)
                nc.vector.reciprocal(r_b[:, :], sd[:, :])
                c_b = sb.tile([P, N], F32, tag="c_b", bufs=2)
                nc.vector.scalar_tensor_tensor(c_b[:, :], m_b[:, :], -1.0, r_b[:, :],
                                               AL.mult, AL.mult)
                return r_b, c_b

            def ln_apply(xin, r_b, c_b):
                zs = []
                for kt in range(DT):
                    t = sb.tile([P, N], F32, tag="lnt", bufs=2)
                    nc.vector.tensor_mul(t[:, :], xin[kt][:, :], r_b[:, :])
                    z = sb.tile([P, N], F32R, tag="z", bufs=4)
                    nc.vector.tensor_add(z[:, :], t[:, :], c_b[:, :])
                    zs.append(z)
                return zs

            def ff_block(xin, w_d, bt, w2bf_d, b2tt):
                """x + 0.5*ff(LN(x)); returns new residual tiles."""
                r_b, c_b = layer_norm_rc(xin)
                zs = ln_apply(xin, r_b, c_b)
                # h = swish(z @ w1 + b1), mt-outer with half-width weight tiles
                h1s = []
                for half in range(2):
                    wts = []
                    for kt in range(DT):
                        wt = sb.tile([P, FF // 2], F32R, tag="wbig", bufs=5)
                        nc.sync.dma_start(
                            wt[:, :], w_d[kt * P:(kt + 1) * P,
                                          half * (FF // 2):(half + 1) * (FF // 2)])
                        wts.append(wt)
                    for mh in range(FT // 2):
                        mt = half * (FT // 2) + mh
                        ph = psp.tile([P, N], F32, tag="acc", bufs=4)
                        for kt in range(DT):
                            nc.tensor.matmul(ph[:, :],
                                             r32(wts[kt][:, mh * P:(mh + 1) * P]),
                                             r32(zs[kt][:, :]),
                                             start=(kt == 0), stop=(kt == DT - 1))
                        sig = sb.tile([P, N], F32, tag="tmp", bufs=3)
                        nc.scalar.activation(sig[:, :], ph[:, :], AF.Sigmoid,
                                             bias=bt[:, mt:mt + 1], scale=1.0)
                        hs = sb.tile([P, N], BF16, tag="h1s", bufs=16)
                        nc.vector.scalar_tensor_tensor(hs[:, :], ph[:, :],
                                                       bt[:, mt:mt + 1], sig[:, :],
                                                       AL.add, AL.mult)
                        h1s.append(hs)
                # y = h @ w2 (bf16), kt-outer with 4 psum accumulators
                pys = [psp.tile([P, N], F32, tag="acc", bufs=4, name=f"pys{i}") for i in range(DT)]
                for kt in range(FT):
                    wt = sb.tile([P, DIM], BF16, tag="wsmb", bufs=6)
                    nc.sync.dma_start(wt[:, :], w2bf_d[kt * P:(kt + 1) * P, :])
                    for mt in range(DT):
                        nc.tensor.matmul(pys[mt][:, :], wt[:, mt * P:(mt + 1) * P],
                                         h1s[kt][:, :],
                                         start=(kt == 0), stop=(kt == FT - 1))
                xo = []
                for mt in range(DT):
                    t = sb.tile([P, N], F32R, tag="x", bufs=7)
                    nc.vector.scalar_tensor_tensor(t[:, :], pys[mt][:, :],
                                                   b2tt[:, mt:mt + 1], xin[mt][:, :],
                                                   AL.add, AL.add)
                    xo.append(t)
                return xo

            # ================= ff1 =================
            x1 = ff_block(xs, w1_d, b1t, w2_d, b2t)

            # ================= attention =================
            r_b, c_b = layer_norm_rc(x1)
            zs = ln_apply(x1, r_b, c_b)

            def proj_qk(w_dram, bias_t, tag):
                wts = []
                for kt in range(DT):
                    wt = sb.tile([P, INNER], F32R, tag="wsm", bufs=4)
                    nc.sync.dma_start(wt[:, :], w_dram[kt * P:(kt + 1) * P, :])
                    wts.append(wt)
                outs = []
                for mt in range(DT):
                    pq = psp.tile([P, N], F32, tag="mm", bufs=2)
                    for kt in range(DT):
                        nc.tensor.matmul(pq[:, :], r32(wts[kt][:, mt * P:(mt + 1) * P]),
                                         r32(zs[kt][:, :]),
                                         start=(kt == 0), stop=(kt == DT - 1))
                    qt = sb.tile([P, N], F32R, tag=tag, bufs=4)
                    nc.vector.tensor_scalar(out=qt[:, :], in0=pq[:, :],
                                            scalar1=bias_t[:, mt:mt + 1], scalar2=None,
                                            op0=AL.add)
                    outs.append(qt)
                return outs

            qTs = proj_qk(wq_d, bqt, "qT")
            kTs = proj_qk(wk_d, bkt, "kT")

            # v in time-major layout with a trailing ones column per head
            wvts = []
            for kt in range(DT):
                wt = sb.tile([P, INNER], F32R, tag="wsm", bufs=4)
                nc.sync.dma_start(wt[:, :], wv_d[kt * P:(kt + 1) * P, :])
                wvts.append(wt)
            vext = []
            for jt in range(DT):
                pv = psp.tile([P, N], F32, tag="mm", bufs=2)
                for kt in range(DT):
                    nc.tensor.matmul(pv[:, :], r32(zs[kt][:, jt * P:(jt + 1) * P]),
                                     r32(wvts[kt][:, :]),
                                     start=(kt == 0), stop=(kt == DT - 1))
                vx = sb.tile([P, H * 2 * DH], F32R, tag="vext", bufs=4)
                vw = vx[:, 0:H * 2 * DH].rearrange("p (h c) -> p h c", c=2 * DH)
                nc.vector.scalar_tensor_tensor(
                    vw[:, :, 0:DH],
                    pv[:, :].rearrange("p (h d) -> p h d", h=H), 1.0,
                    bvt[:, :].rearrange("p (h d) -> p h d", h=H),
                    AL.mult, AL.add)
                nc.vector.tensor_copy(
                    vw[:, :, DH:2 * DH],
                    ones_full[:, None, 0:DH].broadcast_to([P, H, DH]))
                vext.append(vx)

            # qr = q @ relT, bounced through DRAM scratch (640-wide windows)
            for h in range(H):
                hb = (h % 2) * DH
                for it in range(DT):
                    lq = qTs[h // 2][hb:hb + DH, it * P:(it + 1) * P]
                    cr0 = 3 * P - P * it
                    pq1 = psp.tile([P, 320], F32, tag="acc", bufs=4)
                    nc.tensor.matmul(pq1[:, :], r32(lq),
                                     r32(relT[hb:hb + DH, cr0:cr0 + 320]),
                                     start=True, stop=True)
                    pq2 = psp.tile([P, 320], F32, tag="acc", bufs=4)
                    nc.tensor.matmul(pq2[:, :], r32(lq),
                                     r32(relT[hb:hb + DH, cr0 + 320:cr0 + 640]),
                                     start=True, stop=True)
                    qt = sb.tile([P, 640], F32R, tag="qt", bufs=2)
                    nc.scalar.copy(qt[:, 0:320], pq1[:, :])
                    nc.vector.tensor_copy(qt[:, 320:640], pq2[:, :])
                    nc.scalar.dma_start(
                        qr_d[h, it * P:(it + 1) * P, cr0:cr0 + 640], qt[:, :])

            # scores (transposed), softmax over partitions, attn @ v
            oTs = [sb.tile([P, N], F32R, tag="oT", bufs=4, name=f"oTs{i}") for i in range(DT)]
            for h in range(H):
                hb = (h % 2) * DH
                rels = []
                for it in range(DT):
                    rel = sb.tile([P, N], F32R, tag="rel", bufs=5)
                    # contiguous rows: rel_tm[i, j] = qr_rev[i, 512 - i + j]
                    src = bass.AP(qr_d, h * N * QRW + (QRW - 1) * P * it + 4 * P,
                                  [[QRW - 1, P], [1, N]])
                    nc.scalar.dma_start(rel[:, :], src)
                    rels.append(rel)
                exps = []
                for jt in range(DT):
                    pss = psp.tile([P, N], F32, tag="acc", bufs=4)
                    nc.tensor.matmul(pss[:, :],
                                     r32(kTs[h // 2][hb:hb + DH, jt * P:(jt + 1) * P]),
                                     r32(qTs[h // 2][hb:hb + DH, :]),
                                     start=True, stop=False)
                    for it in range(DT):
                        nc.tensor.matmul(
                            pss[:, it * P:(it + 1) * P].bitcast(F32R),
                            rels[it][:, jt * P:(jt + 1) * P], ident[:, :],
                            is_transpose=True, start=False, stop=(it == DT - 1))
                    e = sb.tile([P, N], F32R, tag="exp", bufs=4)
                    nc.scalar.activation(e[:, :], pss[:, :], AF.Exp)
                    exps.append(e)
                po = psp.tile([P, N], F32, tag="mm", bufs=2)
                for jt in range(DT):
                    nc.tensor.matmul(po[:, :],
                                     r32(vext[jt][:, h * 2 * DH:(h + 1) * 2 * DH]),
                                     r32(exps[jt][:, :]),
                                     start=(jt == 0), stop=(jt == DT - 1))
                rb = sb.tile([DH, N], F32, tag="dwt", bufs=3)
                nc.vector.reciprocal(rb[:, :], po[DH:2 * DH, :])
                nc.vector.tensor_mul(oTs[h // 2][hb:hb + DH, :], po[0:DH, :],
                                     rb[:, :])

            # out-projection + residual
            wots = []
            for kt in range(DT):
                wt = sb.tile([P, DIM], F32R, tag="wsm", bufs=4)
                nc.sync.dma_start(wt[:, :], wo_d[kt * P:(kt + 1) * P, :])
                wots.append(wt)
            pas = [psp.tile([P, N], F32, tag="acc", bufs=4, name=f"pas{i}") for i in range(DT)]
            for kt in range(DT):
                for mt in range(DT):
                    nc.tensor.matmul(pas[mt][:, :], r32(wots[kt][:, mt * P:(mt + 1) * P]),
                                     r32(oTs[kt][:, :]),
                                     start=(kt == 0), stop=(kt == DT - 1))
            x2 = []
            for mt in range(DT):
                t = sb.tile([P, N], F32R, tag="x", bufs=7)
                nc.vector.scalar_tensor_tensor(t[:, :], pas[mt][:, :],
                                               bot[:, mt:mt + 1], x1[mt][:, :],
                                               AL.add, AL.add)
                x2.append(t)

            # ================= conv module =================
            glus = []
            for half in range(2):
                c1ts = []
                for kt in range(DT):
                    wt = sb.tile([P, CIN], F32R, tag="wbig", bufs=5)
                    nc.sync.dma_start(
                        wt[:, :], c1_d[kt * P:(kt + 1) * P,
                                       half * CIN:(half + 1) * CIN])
                    c1ts.append(wt)
                for ch in range(CT // 2):
                    ct = half * (CT // 2) + ch
                    pa = psp.tile([P, N], F32, tag="acc", bufs=4)
                    pg = psp.tile([P, N], F32, tag="acc", bufs=4)
                    for kt in range(DT):
                        nc.tensor.matmul(pa[:, :], r32(c1ts[kt][:, ch * P:ch * P + P]),
                                         r32(x2[kt][:, :]),
                                         start=(kt == 0), stop=(kt == DT - 1))
                    for kt in range(DT):
                        nc.tensor.matmul(pg[:, :],
                                         r32(c1ts[kt][:, (CT // 2 + ch) * P:
                                                      (CT // 2 + ch) * P + P]),
                                         r32(x2[kt][:, :]),
                                         start=(kt == 0), stop=(kt == DT - 1))
                    sig = sb.tile([P, N], F32, tag="tmp", bufs=3)
                    nc.scalar.activation(sig[:, :], pg[:, :], AF.Sigmoid,
                                         bias=c1gt[:, ct:ct + 1], scale=1.0)
                    glu = sb.tile([P, PAD + N], BF16, tag="glu", bufs=4)
                    nc.vector.memset(glu[:, 0:PAD], 0.0)
                    nc.vector.scalar_tensor_tensor(glu[:, PAD:PAD + N], pa[:, :],
                                                   c1at[:, ct:ct + 1], sig[:, :],
                                                   AL.add, AL.mult)
                    glus.append(glu)

            # wait: c1 column order is [a(0:1024) | g(1024:2048)]; with the
            # half-split above, half 0 columns 0:1024 are a(ct=0..7), half 1 is g.
            # Reorder handled on host: c1 is passed pre-interleaved per half.

            # depthwise conv as 31 diagonal matmuls per channel block
            hcs = []
            for ct in range(CT):
                dg = sb.tile([P, KW * P], BF16, tag="dg", bufs=2)
                nc.sync.dma_start(dg[:, :], dwd_d[ct, :, :])
                pd = psp.tile([P, N], F32, tag="mm", bufs=2)
                for k in range(KW):
                    nc.tensor.matmul(pd[:, :], dg[:, k * P:(k + 1) * P],
                                     glus[ct][:, k:k + N],
                                     start=(k == 0), stop=(k == KW - 1))
                sig = sb.tile([P, N], F32, tag="dwt", bufs=3)
                nc.scalar.activation(sig[:, :], pd[:, :], AF.Sigmoid,
                                     bias=bntt[:, ct:ct + 1], scale=bnst[:, ct:ct + 1])
                u = sb.tile([P, N], F32, tag="dwt", bufs=3)
                nc.vector.tensor_scalar(out=u[:, :], in0=pd[:, :],
                                        scalar1=bnst[:, ct:ct + 1],
                                        scalar2=bntt[:, ct:ct + 1],
                                        op0=AL.mult, op1=AL.add)
                hc = sb.tile([P, N], F32R, tag="hc", bufs=6)
                nc.vector.tensor_mul(hc[:, :], u[:, :], sig[:, :])
                hcs.append(hc)

            # conv2 + residual (kt-outer)
            pcs = [psp.tile([P, N], F32, tag="acc", bufs=4, name=f"pcs{i}") for i in range(DT)]
            for kt in range(CT):
                wt = sb.tile([P, DIM], F32R, tag="wsm", bufs=4)
                nc.sync.dma_start(wt[:, :], c2_d[kt * P:(kt + 1) * P, :])
                for mt in range(DT):
                    nc.tensor.matmul(pcs[mt][:, :], r32(wt[:, mt * P:(mt + 1) * P]),
                                     r32(hcs[kt][:, :]),
                                     start=(kt == 0), stop=(kt == CT - 1))
            x3 = []
            for mt in range(DT):
                t = sb.tile([P, N], F32R, tag="x", bufs=7)
                nc.vector.scalar_tensor_tensor(t[:, :], pcs[mt][:, :],
                                               c2bt[:, mt:mt + 1], x2[mt][:, :],
                                               AL.add, AL.add)
                x3.append(t)

            # ================= ff2 =================
            x4 = ff_block(x3, w3_d, b3t, w4_d, b4t)

            # ================= post-LN =================
            r_b, c_b = layer_norm_rc(x4)
            for mt in range(DT):
                t = sb.tile([P, N], F32, tag="lnt", bufs=2)
                nc.vector.tensor_mul(t[:, :], x4[mt][:, :], r_b[:, :])
                t2 = sb.tile([P, N], F32, tag="lnt2", bufs=1)
                nc.vector.tensor_add(t2[:, :], t[:, :], c_b[:, :])
                ot = sb.tile([P, N], F32, tag="outt", bufs=2)
                nc.vector.tensor_scalar(out=ot[:, :], in0=t2[:, :],
                                        scalar1=pngt[:, mt:mt + 1],
                                        scalar2=pnbt[:, mt:mt + 1],
                                        op0=AL.mult, op1=AL.add)
                nc.scalar.dma_start(outT_d[mt * P:(mt + 1) * P, :], ot[:, :])

    if split_waits:
        _split_matmul_waits(nc, mybir)
    return nc


def _split_matmul_waits(nc, mybir):
    """This walrus build rejects engine instructions carrying more than one
    sync wait; hoist the extras onto EventSemaphore instructions on the same
    engine queue right before the instruction."""
    fn = nc.m.functions[0]
    ctr = 0
    for blk in fn.blocks:
        out = []
        changed = False
        for ins in blk.instructions:
            si = ins.sync_info
            if (si is not None and si.on_wait and len(si.on_wait) > 1
                    and not isinstance(ins, (mybir.InstEventSemaphore,
                                             mybir.InstNoOp))):
                waits = list(si.on_wait)
                for w in waits[:-1]:
                    ev = mybir.InstNoOp(
                        name=f"EVW-{ctr}", ins=[], outs=[],
                        sync_info=mybir.SyncInfo(on_wait=[w], on_update=[]))
                    ev.engine = ins.engine
                    ctr += 1
                    out.append(ev)
                ins.sync_info = mybir.SyncInfo(
                    on_wait=[waits[-1]], on_update=list(si.on_update or []))
                changed = True
            out.append(ins)
        if changed:
            blk.instructions = out


def prep_inputs(inputs):
    """Host-side preprocessing: fold LN affines / scales / biases into weights."""
    import ml_dtypes

    f = np.float32
    bf = ml_dtypes.bfloat16
    ii = {k: np.asarray(v, dtype=f) for k, v in inputs.items()}

    def colmaj(b, nb):
        return np.ascontiguousarray(b.astype(f).reshape(nb, P).T)

    g1, be1 = ii["ff1_ln_g"], ii["ff1_ln_b"]
    w1 = np.ascontiguousarray(g1[:, None] * ii["ff1_w1"])
    b1 = colmaj(be1 @ ii["ff1_w1"] + ii["ff1_b1"], FT)
    w2 = np.ascontiguousarray((0.5 * ii["ff1_w2"]).astype(bf))
    b2 = colmaj(0.5 * ii["ff1_b2"], DT)

    ag, ab = ii["attn_ln_g"], ii["attn_ln_b"]
    sc = DH ** -0.5
    wq = np.ascontiguousarray(ag[:, None] * ii["wq"] * sc)
    bq = colmaj((ab @ ii["wq"] + ii["bq"]) * sc, DT)
    wkv, bkv = ii["wkv"], ii["bkv"]
    wk = np.ascontiguousarray(ag[:, None] * wkv[:, :INNER])
    bk = colmaj(ab @ wkv[:, :INNER] + bkv[:INNER], DT)
    wv = np.ascontiguousarray(ag[:, None] * wkv[:, INNER:])
    bv = np.ascontiguousarray(np.broadcast_to(
        ab @ wkv[:, INNER:] + bkv[INNER:], (P, INNER)))
    wo = np.ascontiguousarray(ii["wo"])
    bo = colmaj(ii["bo"], DT)
    # relT rows: head feature d lives at partition (h%2)*64 + d -> duplicate rows
    rT = ii["rel_emb"].T[:, ::-1]  # [64, 1025] column-reversed
    relT = np.ascontiguousarray(np.concatenate([rT, rT], axis=0))  # [128, 1025]

    # c1 columns reordered to match the kernel's half-split loop:
    # half h covers channel blocks ct=4h..4h+3 and lays out [a-cols | g-cols]
    w = ii["conv1_w"]
    HC = CIN // 2  # 512
    c1 = np.ascontiguousarray(np.concatenate(
        [w[:, 0:HC], w[:, CIN:CIN + HC], w[:, HC:CIN], w[:, CIN + HC:]], axis=1))
    c1b = ii["conv1_b"]
    c1a = colmaj(c1b[:CIN], CT)
    c1g = colmaj(c1b[CIN:], CT)
    dwd = np.zeros((CT, P, KW, P), dtype=bf)
    wr = ii["dw_w"].reshape(CT, P, KW).astype(bf)
    pp = np.arange(P)
    for ct in range(CT):
        for k in range(KW):
            dwd[ct, pp, k, pp] = wr[ct, :, k]
    dwd = np.ascontiguousarray(dwd.reshape(CT, P, KW * P))
    inv = 1.0 / np.sqrt(ii["bn_var"] + EPS)
    s = inv * ii["bn_g"]
    t = ii["bn_b"] - ii["bn_mean"] * s
    bns = colmaj(s, CT)
    bnt = colmaj(t + s * ii["dw_b"], CT)
    c2 = np.ascontiguousarray(ii["conv2_w"])
    c2b = colmaj(ii["conv2_b"], DT)

    g3, be3 = ii["ff2_ln_g"], ii["ff2_ln_b"]
    w3 = np.ascontiguousarray(g3[:, None] * ii["ff2_w1"])
    b3 = colmaj(be3 @ ii["ff2_w1"] + ii["ff2_b1"], FT)
    w4 = np.ascontiguousarray((0.5 * ii["ff2_w2"]).astype(bf))
    b4 = colmaj(0.5 * ii["ff2_b2"], DT)

    png = colmaj(ii["pn_g"], DT)
    pnb = colmaj(ii["pn_b"], DT)

    shared = dict(w1=w1, b1=b1, w2=w2, b2=b2, wq=wq, bq=bq, wk=wk, bk=bk,
                  wv=wv, bvb=bv, wo=wo, bo=bo, relT=relT, c1=c1, c1a=c1a,
                  c1g=c1g, dwdiag=dwd, bns=bns, bnt=bnt, c2=c2, c2b=c2b,
                  w3=w3, b3=b3, w4=w4, b4=b4, png=png, pnb=pnb,
                  antid=np.ascontiguousarray(np.eye(P, dtype=f)),
                  onesf=np.ones((P, P), dtype=f))
    x = ii["x"]
    in_maps = []
    for b in range(NCORES):
        m = dict(shared)
        m["xT"] = np.ascontiguousarray(x[b].T)
        in_maps.append(m)
    return in_maps


_BUILT = None


def run(inputs, trace=False):
    global _BUILT
    from concourse import bass_utils

    in_maps = prep_inputs(inputs)
    if _BUILT is None:
        _BUILT = build()
    res = bass_utils.run_bass_kernel_spmd(
        _BUILT, in_maps, core_ids=list(range(NCORES)), trace=trace)
    out = np.stack([np.asarray(r["outT"]).T for r in res.results])
    return np.ascontiguousarray(out.astype(np.float32)), res


def kernel(**inputs):
    out, _ = run(inputs, trace=False)
    return out


# revision 25
# speedup vs baseline: 8.4562x; 1.0208x over previous
"""Trainium2 Bass kernel: Conformer block (B=8, N=512, DIM=512, H=8, DH=64, FF=2048, CIN=1024, K=31).

Sharding: pure data-parallel over batch — each of the 8 NeuronCores processes one
batch item with the full weight set (no collectives).

Layout: activations are kept FEATURE-major ([feature, time] = x.T) on chip so that
chained matmuls need no transposes (weights stay in natural [din, dout] layout as
the stationary operand).  LayerNorm reductions over features become ones-vector
matmuls on the PE; per-time-step affine factors are broadcast across partitions
with a GPSIMD partition_broadcast.

Relative-position attention uses the shift-gather trick: qr = q @ rel_emb.T is
bounced through an internal DRAM scratch and read back with a strided
(stride = row+1, step -1) access pattern so that rel[j, i] = qr[i, i-j+512]
lands directly as the transposed score tile.  Scores are computed transposed
(dots_T[j, i]) so softmax runs over the partition axis: exp on ACT, the
denominator via a ones-column fused into the attn@v matmul, and the final
normalization as a broadcasted multiply.

The causal depthwise conv runs on the PE as 31 PSUM-accumulated matmuls per
128-channel block against diagonal stationary matrices; the diagonals are
(re)written with a single strided DMA per block (dst step = row+1).

Matmuls use float32r (1 cycle/row for N>=256); the FFN second matmul and the
depthwise conv run in bf16.
"""

import sys

for _p in ("/opt/trn_rl_repo", "/root/.axon_site/_ro/trn_rl_repo"):
    if _p not in sys.path:
        sys.path.insert(0, _p)

import numpy as np

B, N, DIM, H, DH, MULT, EXP, KW, MAXP = 8, 512, 512, 8, 64, 4, 2, 31, 512
INNER = H * DH
FF = DIM * MULT
CIN = DIM * EXP
EPS = 1e-5
P = 128
DT = DIM // P      # 4  feature tiles of the residual stream
FT = FF // P       # 16 ff hidden tiles
CT = CIN // P      # 8  conv channel tiles
NCORES = 8
PAD = KW - 1       # 30 causal pad


def build(split_waits=True):
    """Build the single-core Bass module (SPMD: same NEFF on all 8 cores)."""
    import concourse.bass as bass
    import concourse.mybir as mybir
    import concourse.tile as tile

    F32 = mybir.dt.float32
    F32R = mybir.dt.float32r
    BF16 = mybir.dt.bfloat16
    AF = mybir.ActivationFunctionType
    AL = mybir.AluOpType

    nc = bass.Bass()

    # ---------------- I/O ----------------
    xT_d = nc.dram_tensor("xT", [DIM, N], F32R, kind="ExternalInput")
    w1_d = nc.dram_tensor("w1", [DIM, FF], F32R, kind="ExternalInput")
    b1_d = nc.dram_tensor("b1", [P, FT], F32, kind="ExternalInput")
    w2_d = nc.dram_tensor("w2", [FF, DIM], BF16, kind="ExternalInput")
    b2_d = nc.dram_tensor("b2", [P, DT], F32, kind="ExternalInput")
    wq_d = nc.dram_tensor("wq", [DIM, INNER], F32R, kind="ExternalInput")
    bq_d = nc.dram_tensor("bq", [P, DT], F32, kind="ExternalInput")
    wk_d = nc.dram_tensor("wk", [DIM, INNER], F32R, kind="ExternalInput")
    bk_d = nc.dram_tensor("bk", [P, DT], F32, kind="ExternalInput")
    wv_d = nc.dram_tensor("wv", [DIM, INNER], F32R, kind="ExternalInput")
    bv_d = nc.dram_tensor("bvb", [P, INNER], F32R, kind="ExternalInput")
    wo_d = nc.dram_tensor("wo", [INNER, DIM], F32R, kind="ExternalInput")
    bo_d = nc.dram_tensor("bo", [P, DT], F32, kind="ExternalInput")
    relT_d = nc.dram_tensor("relT", [P, 2 * MAXP + 1], F32R, kind="ExternalInput")
    c1_d = nc.dram_tensor("c1", [DIM, 2 * CIN], F32R, kind="ExternalInput")
    c1a_d = nc.dram_tensor("c1a", [P, CT], F32, kind="ExternalInput")
    c1g_d = nc.dram_tensor("c1g", [P, CT], F32, kind="ExternalInput")
    dwd_d = nc.dram_tensor("dwdiag", [CT, P, KW * P], BF16, kind="ExternalInput")
    bns_d = nc.dram_tensor("bns", [P, CT], F32, kind="ExternalInput")
    bnt_d = nc.dram_tensor("bnt", [P, CT], F32, kind="ExternalInput")
    c2_d = nc.dram_tensor("c2", [CIN, DIM], F32R, kind="ExternalInput")
    c2b_d = nc.dram_tensor("c2b", [P, DT], F32, kind="ExternalInput")
    w3_d = nc.dram_tensor("w3", [DIM, FF], F32R, kind="ExternalInput")
    b3_d = nc.dram_tensor("b3", [P, FT], F32, kind="ExternalInput")
    w4_d = nc.dram_tensor("w4", [FF, DIM], BF16, kind="ExternalInput")
    b4_d = nc.dram_tensor("b4", [P, DT], F32, kind="ExternalInput")
    png_d = nc.dram_tensor("png", [P, DT], F32, kind="ExternalInput")
    pnb_d = nc.dram_tensor("pnb", [P, DT], F32, kind="ExternalInput")
    antid_d = nc.dram_tensor("antid", [P, P], F32R, kind="ExternalInput")
    onesf_d = nc.dram_tensor("onesf", [P, P], F32R, kind="ExternalInput")

    outT_d = nc.dram_tensor("outT", [DIM, N], F32, kind="ExternalOutput")

    QRW = 2 * MAXP + 1  # 1025 scratch row width
    qr_d = nc.dram_tensor("qr_scratch", [H, N, QRW], F32R, kind="Internal")

    def r32(ap):
        return ap.bitcast(F32R)

    with tile.TileContext(nc) as tc:
        with (
            nc.allow_low_precision(reason="fp32r/bf16 matmul feeds"),
            tc.tile_pool(name="cst", bufs=1) as cst,
            tc.tile_pool(name="sb", bufs=2) as sb,
            tc.tile_pool(name="ps", bufs=2, space="PSUM") as psp,
        ):

            # ---------------- constants ----------------
            ones_full = cst.tile([P, P], F32R, tag="ones_full")
            nc.sync.dma_start(ones_full[:, :], onesf_d[:, :])
            ident = cst.tile([P, P], F32R, tag="ident")
            nc.sync.dma_start(ident[:, :], antid_d[:, :])
            relT = cst.tile([P, QRW], F32R, tag="relT")
            nc.sync.dma_start(relT[:, :], relT_d[:, :])
            b1t = cst.tile([P, FT], F32, tag="b1t")
            nc.sync.dma_start(b1t[:, :], b1_d[:, :])
            b2t = cst.tile([P, DT], F32, tag="b2t")
            nc.sync.dma_start(b2t[:, :], b2_d[:, :])
            bqt = cst.tile([P, DT], F32, tag="bqt")
            nc.sync.dma_start(bqt[:, :], bq_d[:, :])
            bkt = cst.tile([P, DT], F32, tag="bkt")
            nc.sync.dma_start(bkt[:, :], bk_d[:, :])
            bvt = cst.tile([P, INNER], F32R, tag="bvt")
            nc.sync.dma_start(bvt[:, :], bv_d[:, :])
            bot = cst.tile([P, DT], F32, tag="bot")
            nc.sync.dma_start(bot[:, :], bo_d[:, :])
            c1at = cst.tile([P, CT], F32, tag="c1at")
            nc.sync.dma_start(c1at[:, :], c1a_d[:, :])
            c1gt = cst.tile([P, CT], F32, tag="c1gt")
            nc.sync.dma_start(c1gt[:, :], c1g_d[:, :])
            bnst = cst.tile([P, CT], F32, tag="bnst")
            nc.sync.dma_start(bnst[:, :], bns_d[:, :])
            bntt = cst.tile([P, CT], F32, tag="bntt")
            nc.sync.dma_start(bntt[:, :], bnt_d[:, :])
            c2bt = cst.tile([P, DT], F32, tag="c2bt")
            nc.sync.dma_start(c2bt[:, :], c2b_d[:, :])
            b3t = cst.tile([P, FT], F32, tag="b3t")
            nc.sync.dma_start(b3t[:, :], b3_d[:, :])
            b4t = cst.tile([P, DT], F32, tag="b4t")
            nc.sync.dma_start(b4t[:, :], b4_d[:, :])
            pngt = cst.tile([P, DT], F32, tag="pngt")
            nc.sync.dma_start(pngt[:, :], png_d[:, :])
            pnbt = cst.tile([P, DT], F32, tag="pnbt")
            nc.sync.dma_start(pnbt[:, :], pnb_d[:, :])

            # ---------------- load x (already transposed on host) ----------------
            xs = []
            for mt in range(DT):
                xt = sb.tile([P, N], F32R, tag="x", bufs=7)
                nc.sync.dma_start(xt[:, :], xT_d[mt * P:(mt + 1) * P, :])
                xs.append(xt)

            # ---------------- helpers ----------------
            def layer_norm_rc(xin):
                """Stats of LN over the partition (feature) axis.

                Returns r_b, c_b [128, 512] tiles with z = x*r_b + c_b."""
                ps_sum = psp.tile([P, N], F32, tag="s1", bufs=1)
                for kt in range(DT):
                    nc.tensor.matmul(ps_sum[:, :], ones_full[:, :], xin[kt][:, :],
                                     start=(kt == 0), stop=(kt == DT - 1))
                ps_sq = psp.tile([P, N], F32, tag="s2", bufs=1)
                for kt in range(DT):
                    xsq = sb.tile([P, N], F32R, tag="tmp", bufs=3)
                    nc.scalar.square(xsq[:, :], xin[kt][:, :])
                    nc.tensor.matmul(ps_sq[:, :], ones_full[:, :], xsq[:, :],
                                     start=(kt == 0), stop=(kt == DT - 1))
                m_b = sb.tile([P, N], F32, tag="mtile", bufs=1)
                nc.vector.tensor_scalar(out=m_b[:, :], in0=ps_sum[:, :],
                                        scalar1=1.0 / DIM, scalar2=None, op0=AL.mult)
                q_b = sb.tile([P, N], F32, tag="tmp", bufs=3)
                nc.scalar.mul(q_b[:, :], ps_sq[:, :], 1.0 / DIM)
                nm2 = sb.tile([P, N], F32, tag="tmp", bufs=3)
                nc.vector.scalar_tensor_tensor(nm2[:, :], m_b[:, :], -1.0, m_b[:, :],
                                               AL.mult, AL.mult)
                veps = sb.tile([P, N], F32, tag="tmp", bufs=3)
                nc.vector.scalar_tensor_tensor(veps[:, :], q_b[:, :], EPS, nm2[:, :],
                                               AL.add, AL.add)
                lnv = sb.tile([P, N], F32, tag="tmp", bufs=3)
                nc.scalar.activation(lnv[:, :], veps[:, :], AF.Ln)
                r_b = sb.tile([P, N], F32, tag="r_b", bufs=2)
                nc.scalar.activation(r_b[:, :], lnv[:, :], AF.Exp, scale=-0.5)
                c_b = sb.tile([P, N], F32, tag="c_b", bufs=2)
                nc.vector.scalar_tensor_tensor(c_b[:, :], m_b[:, :], -1.0, r_b[:, :],
                                               AL.mult, AL.mult)
                return r_b, c_b

            def ln_apply(xin, r_b, c_b):
                zs = []
                for kt in range(DT):
                    t = sb.tile([P, N], F32, tag="lnt", bufs=2)
                    nc.vector.tensor_mul(t[:, :], xin[kt][:, :], r_b[:, :])
                    z = sb.tile([P, N], F32R, tag="z", bufs=4)
                    nc.vector.tensor_add(z[:, :], t[:, :], c_b[:, :])
                    zs.append(z)
                return zs

            def ff_block(xin, w_d, bt, w2bf_d, b2tt):
                """x + 0.5*ff(LN(x)); returns new residual tiles."""
                r_b, c_b = layer_norm_rc(xin)
                zs = ln_apply(xin, r_b, c_b)
                # h = swish(z @ w1 + b1), mt-outer with half-width weight tiles
                h1s = []
                for half in range(2):
                    wts = []
                    for kt in range(DT):
                        wt = sb.tile([P, FF // 2], F32R, tag="wbig", bufs=5)
                        nc.sync.dma_start(
                            wt[:, :], w_d[kt * P:(kt + 1) * P,
                                          half * (FF // 2):(half + 1) * (FF // 2)])
                        wts.append(wt)
                    for mh in range(FT // 2):
                        mt = half * (FT // 2) + mh
                        ph = psp.tile([P, N], F32, tag="acc", bufs=4)
                        for kt in range(DT):
                            nc.tensor.matmul(ph[:, :],
                                             r32(wts[kt][:, mh * P:(mh + 1) * P]),
                                             r32(zs[kt][:, :]),
                                             start=(kt == 0), stop=(kt == DT - 1))
                        sig = sb.tile([P, N], F32, tag="tmp", bufs=3)
                        nc.scalar.activation(sig[:, :], ph[:, :], AF.Sigmoid,
                                             bias=bt[:, mt:mt + 1], scale=1.0)
                        hs = sb.tile([P, N], BF16, tag="h1s", bufs=16)
                        nc.vector.scalar_tensor_tensor(hs[:, :], ph[:, :],
                                                       bt[:, mt:mt + 1], sig[:, :],
                                                       AL.add, AL.mult)
                        h1s.append(hs)
                # y = h @ w2 (bf16), kt-outer with 4 psum accumulators
                pys = [psp.tile([P, N], F32, tag="acc", bufs=4, name=f"pys{i}") for i in range(DT)]
                for kt in range(FT):
                    wt = sb.tile([P, DIM], BF16, tag="wsmb", bufs=6)
                    nc.sync.dma_start(wt[:, :], w2bf_d[kt * P:(kt + 1) * P, :])
                    for mt in range(DT):
                        nc.tensor.matmul(pys[mt][:, :], wt[:, mt * P:(mt + 1) * P],
                                         h1s[kt][:, :],
                                         start=(kt == 0), stop=(kt == FT - 1))
                xo = []
                for mt in range(DT):
                    t = sb.tile([P, N], F32R, tag="x", bufs=7)
                    nc.vector.scalar_tensor_tensor(t[:, :], pys[mt][:, :],
                                                   b2tt[:, mt:mt + 1], xin[mt][:, :],
                                                   AL.add, AL.add)
                    xo.append(t)
                return xo

            # ================= ff1 =================
            x1 = ff_block(xs, w1_d, b1t, w2_d, b2t)

            # ================= attention =================
            r_b, c_b = layer_norm_rc(x1)
            zs = ln_apply(x1, r_b, c_b)

            def proj_qk(w_dram, bias_t, tag):
                wts = []
                for kt in range(DT):
                    wt = sb.tile([P, INNER], F32R, tag="wsm", bufs=4)
                    nc.sync.dma_start(wt[:, :], w_dram[kt * P:(kt + 1) * P, :])
                    wts.append(wt)
                outs = []
                for mt in range(DT):
                    pq = psp.tile([P, N], F32, tag="mm", bufs=2)
                    for kt in range(DT):
                        nc.tensor.matmul(pq[:, :], r32(wts[kt][:, mt * P:(mt + 1) * P]),
                                         r32(zs[kt][:, :]),
                                         start=(kt == 0), stop=(kt == DT - 1))
                    qt = sb.tile([P, N], F32R, tag=tag, bufs=4)
                    nc.vector.tensor_scalar(out=qt[:, :], in0=pq[:, :],
                                            scalar1=bias_t[:, mt:mt + 1], scalar2=None,
                                            op0=AL.add)
                    outs.append(qt)
                return outs

            qTs = proj_qk(wq_d, bqt, "qT")
            kTs = proj_qk(wk_d, bkt, "kT")

            # v in time-major layout with a trailing ones column per head
            wvts = []
            for kt in range(DT):
                wt = sb.tile([P, INNER], F32R, tag="wsm", bufs=4)
                nc.sync.dma_start(wt[:, :], wv_d[kt * P:(kt + 1) * P, :])
                wvts.append(wt)
            vext = []
            for jt in range(DT):
                pv = psp.tile([P, N], F32, tag="mm", bufs=2)
                for kt in range(DT):
                    nc.tensor.matmul(pv[:, :], r32(zs[kt][:, jt * P:(jt + 1) * P]),
                                     r32(wvts[kt][:, :]),
                                     start=(kt == 0), stop=(kt == DT - 1))
                vx = sb.tile([P, H * 2 * DH], F32R, tag="vext", bufs=4)
                vw = vx[:, 0:H * 2 * DH].rearrange("p (h c) -> p h c", c=2 * DH)
                nc.vector.scalar_tensor_tensor(
                    vw[:, :, 0:DH],
                    pv[:, :].rearrange("p (h d) -> p h d", h=H), 1.0,
                    bvt[:, :].rearrange("p (h d) -> p h d", h=H),
                    AL.mult, AL.add)
                nc.vector.tensor_copy(
                    vw[:, :, DH:2 * DH],
                    ones_full[:, None, 0:DH].broadcast_to([P, H, DH]))
                vext.append(vx)

            # qr = q @ relT, bounced through DRAM scratch (640-wide windows)
            for h in range(H):
                hb = (h % 2) * DH
                for it in range(DT):
                    lq = qTs[h // 2][hb:hb + DH, it * P:(it + 1) * P]
                    cr0 = 3 * P - P * it
                    pq1 = psp.tile([P, 320], F32, tag="acc", bufs=4)
                    nc.tensor.matmul(pq1[:, :], r32(lq),
                                     r32(relT[hb:hb + DH, cr0:cr0 + 320]),
                                     start=True, stop=True)
                    pq2 = psp.tile([P, 320], F32, tag="acc", bufs=4)
                    nc.tensor.matmul(pq2[:, :], r32(lq),
                                     r32(relT[hb:hb + DH, cr0 + 320:cr0 + 640]),
                                     start=True, stop=True)
                    qt = sb.tile([P, 640], F32R, tag="qt", bufs=2)
                    nc.scalar.copy(qt[:, 0:320], pq1[:, :])
                    nc.vector.tensor_copy(qt[:, 320:640], pq2[:, :])
                    nc.scalar.dma_start(
                        qr_d[h, it * P:(it + 1) * P, cr0:cr0 + 640], qt[:, :])

            # scores (transposed), softmax over partitions, attn @ v
            oTs = [sb.tile([P, N], F32R, tag="oT", bufs=4, name=f"oTs{i}") for i in range(DT)]
            for h in range(H):
                hb = (h % 2) * DH
                rels = []
                for it in range(DT):
                    rel = sb.tile([P, N], F32R, tag="rel", bufs=5)
                    # contiguous rows: rel_tm[i, j] = qr_rev[i, 512 - i + j]
                    src = bass.AP(qr_d, h * N * QRW + (QRW - 1) * P * it + 4 * P,
                                  [[QRW - 1, P], [1, N]])
                    nc.scalar.dma_start(rel[:, :], src)
                    rels.append(rel)
                exps = []
                for jt in range(DT):
                    pss = psp.tile([P, N], F32, tag="acc", bufs=4)
                    nc.tensor.matmul(pss[:, :],
                                     r32(kTs[h // 2][hb:hb + DH, jt * P:(jt + 1) * P]),
                                     r32(qTs[h // 2][hb:hb + DH, :]),
                                     start=True, stop=False)
                    for it in range(DT):
                        nc.tensor.matmul(
                            pss[:, it * P:(it + 1) * P].bitcast(F32R),
                            rels[it][:, jt * P:(jt + 1) * P], ident[:, :],
                            is_transpose=True, start=False, stop=(it == DT - 1))
                    e = sb.tile([P, N], F32R, tag="exp", bufs=4)
                    nc.scalar.activation(e[:, :], pss[:, :], AF.Exp)
                    exps.append(e)
                po = psp.tile([P, N], F32, tag="mm", bufs=2)
                for jt in range(DT):
                    nc.tensor.matmul(po[:, :],
                                     r32(vext[jt][:, h * 2 * DH:(h + 1) * 2 * DH]),
                                     r32(exps[jt][:, :]),
                                     start=(jt == 0), stop=(jt == DT - 1))
                lnd = sb.tile([DH, N], F32, tag="dwt", bufs=3)
                nc.scalar.activation(lnd[:, :], po[DH:2 * DH, :], AF.Ln)
                rb = sb.tile([DH, N], F32, tag="dwt", bufs=3)
                nc.scalar.activation(rb[:, :], lnd[:, :], AF.Exp, scale=-1.0)
                nc.vector.tensor_mul(oTs[h // 2][hb:hb + DH, :], po[0:DH, :],
                                     rb[:, :])

            # out-projection + residual
            wots = []
            for kt in range(DT):
                wt = sb.tile([P, DIM], F32R, tag="wsm", bufs=4)
                nc.sync.dma_start(wt[:, :], wo_d[kt * P:(kt + 1) * P, :])
                wots.append(wt)
            pas = [psp.tile([P, N], F32, tag="acc", bufs=4, name=f"pas{i}") for i in range(DT)]
            for kt in range(DT):
                for mt in range(DT):
                    nc.tensor.matmul(pas[mt][:, :], r32(wots[kt][:, mt * P:(mt + 1) * P]),
                                     r32(oTs[kt][:, :]),
                                     start=(kt == 0), stop=(kt == DT - 1))
            x2 = []
            for mt in range(DT):
                t = sb.tile([P, N], F32R, tag="x", bufs=7)
                nc.vector.scalar_tensor_tensor(t[:, :], pas[mt][:, :],
                                               bot[:, mt:mt + 1], x1[mt][:, :],
                                               AL.add, AL.add)
                x2.append(t)

            # ================= conv module =================
            glus = []
            for half in range(2):
                c1ts = []
                for kt in range(DT):
                    wt = sb.tile([P, CIN], F32R, tag="wbig", bufs=5)
                    nc.sync.dma_start(
                        wt[:, :], c1_d[kt * P:(kt + 1) * P,
                                       half * CIN:(half + 1) * CIN])
                    c1ts.append(wt)
                for ch in range(CT // 2):
                    ct = half * (CT // 2) + ch
                    pa = psp.tile([P, N], F32, tag="acc", bufs=4)
                    pg = psp.tile([P, N], F32, tag="acc", bufs=4)
                    for kt in range(DT):
                        nc.tensor.matmul(pa[:, :], r32(c1ts[kt][:, ch * P:ch * P + P]),
                                         r32(x2[kt][:, :]),
                                         start=(kt == 0), stop=(kt == DT - 1))
                    for kt in range(DT):
                        nc.tensor.matmul(pg[:, :],
                                         r32(c1ts[kt][:, (CT // 2 + ch) * P:
                                                      (CT // 2 + ch) * P + P]),
                                         r32(x2[kt][:, :]),
                                         start=(kt == 0), stop=(kt == DT - 1))
                    sig = sb.tile([P, N], F32, tag="tmp", bufs=3)
                    nc.scalar.activation(sig[:, :], pg[:, :], AF.Sigmoid,
                                         bias=c1gt[:, ct:ct + 1], scale=1.0)
                    glu = sb.tile([P, PAD + N], BF16, tag="glu", bufs=4)
                    nc.vector.memset(glu[:, 0:PAD], 0.0)
                    nc.vector.scalar_tensor_tensor(glu[:, PAD:PAD + N], pa[:, :],
                                                   c1at[:, ct:ct + 1], sig[:, :],
                                                   AL.add, AL.mult)
                    glus.append(glu)

            # wait: c1 column order is [a(0:1024) | g(1024:2048)]; with the
            # half-split above, half 0 columns 0:1024 are a(ct=0..7), half 1 is g.
            # Reorder handled on host: c1 is passed pre-interleaved per half.

            # depthwise conv as 31 diagonal matmuls per channel block
            hcs = []
            for ct in range(CT):
                dg = sb.tile([P, KW * P], BF16, tag="dg", bufs=2)
                nc.sync.dma_start(dg[:, :], dwd_d[ct, :, :])
                pd = psp.tile([P, N], F32, tag="mm", bufs=2)
                for k in range(KW):
                    nc.tensor.matmul(pd[:, :], dg[:, k * P:(k + 1) * P],
                                     glus[ct][:, k:k + N],
                                     start=(k == 0), stop=(k == KW - 1))
                sig = sb.tile([P, N], F32, tag="dwt", bufs=3)
                nc.scalar.activation(sig[:, :], pd[:, :], AF.Sigmoid,
                                     bias=bntt[:, ct:ct + 1], scale=bnst[:, ct:ct + 1])
                u = sb.tile([P, N], F32, tag="dwt", bufs=3)
                nc.vector.tensor_scalar(out=u[:, :], in0=pd[:, :],
                                        scalar1=bnst[:, ct:ct + 1],
                                        scalar2=bntt[:, ct:ct + 1],
                                        op0=AL.mult, op1=AL.add)
                hc = sb.tile([P, N], F32R, tag="hc", bufs=6)
                nc.vector.tensor_mul(hc[:, :], u[:, :], sig[:, :])
                hcs.append(hc)

            # conv2 + residual (kt-outer)
            pcs = [psp.tile([P, N], F32, tag="acc", bufs=4, name=f"pcs{i}") for i in range(DT)]
            for kt in range(CT):
                wt = sb.tile([P, DIM], F32R, tag="wsm", bufs=4)
                nc.sync.dma_start(wt[:, :], c2_d[kt * P:(kt + 1) * P, :])
                for mt in range(DT):
                    nc.tensor.matmul(pcs[mt][:, :], r32(wt[:, mt * P:(mt + 1) * P]),
                                     r32(hcs[kt][:, :]),
                                     start=(kt == 0), stop=(kt == CT - 1))
            x3 = []
            for mt in range(DT):
                t = sb.tile([P, N], F32R, tag="x", bufs=7)
                nc.vector.scalar_tensor_tensor(t[:, :], pcs[mt][:, :],
                                               c2bt[:, mt:mt + 1], x2[mt][:, :],
                                               AL.add, AL.add)
                x3.append(t)

            # ================= ff2 =================
            x4 = ff_block(x3, w3_d, b3t, w4_d, b4t)

            # ================= post-LN =================
            r_b, c_b = layer_norm_rc(x4)
            for mt in range(DT):
                t = sb.tile([P, N], F32, tag="lnt", bufs=2)
                nc.vector.tensor_mul(t[:, :], x4[mt][:, :], r_b[:, :])
                t2 = sb.tile([P, N], F32, tag="lnt2", bufs=1)
                nc.vector.tensor_add(t2[:, :], t[:, :], c_b[:, :])
                ot = sb.tile([P, N], F32, tag="outt", bufs=2)
                nc.vector.tensor_scalar(out=ot[:, :], in0=t2[:, :],
                                        scalar1=pngt[:, mt:mt + 1],
                                        scalar2=pnbt[:, mt:mt + 1],
                                        op0=AL.mult, op1=AL.add)
                nc.scalar.dma_start(outT_d[mt * P:(mt + 1) * P, :], ot[:, :])

    if split_waits:
        _split_matmul_waits(nc, mybir)
    return nc


def _split_matmul_waits(nc, mybir):
    """This walrus build rejects engine instructions carrying more than one
    sync wait; hoist the extras onto EventSemaphore instructions on the same
    engine queue right before the instruction."""
    fn = nc.m.functions[0]
    ctr = 0
    for blk in fn.blocks:
        out = []
        changed = False
        for ins in blk.instructions:
            si = ins.sync_info
            if (si is not None and si.on_wait and len(si.on_wait) > 1
                    and not isinstance(ins, (mybir.InstEventSemaphore,
                                             mybir.InstNoOp))):
                waits = list(si.on_wait)
                for w in waits[:-1]:
                    ev = mybir.InstNoOp(
                        name=f"EVW-{ctr}", ins=[], outs=[],
                        sync_info=mybir.SyncInfo(on_wait=[w], on_update=[]))
                    ev.engine = ins.engine
                    ctr += 1
                    out.append(ev)
                ins.sync_info = mybir.SyncInfo(
                    on_wait=[waits[-1]], on_update=list(si.on_update or []))
                changed = True
            out.append(ins)
        if changed:
            blk.instructions = out


def prep_inputs(inputs):
    """Host-side preprocessing: fold LN affines / scales / biases into weights."""
    import ml_dtypes

    f = np.float32
    bf = ml_dtypes.bfloat16
    ii = {k: np.asarray(v, dtype=f) for k, v in inputs.items()}

    def colmaj(b, nb):
        return np.ascontiguousarray(b.astype(f).reshape(nb, P).T)

    g1, be1 = ii["ff1_ln_g"], ii["ff1_ln_b"]
    w1 = np.ascontiguousarray(g1[:, None] * ii["ff1_w1"])
    b1 = colmaj(be1 @ ii["ff1_w1"] + ii["ff1_b1"], FT)
    w2 = np.ascontiguousarray((0.5 * ii["ff1_w2"]).astype(bf))
    b2 = colmaj(0.5 * ii["ff1_b2"], DT)

    ag, ab = ii["attn_ln_g"], ii["attn_ln_b"]
    sc = DH ** -0.5
    wq = np.ascontiguousarray(ag[:, None] * ii["wq"] * sc)
    bq = colmaj((ab @ ii["wq"] + ii["bq"]) * sc, DT)
    wkv, bkv = ii["wkv"], ii["bkv"]
    wk = np.ascontiguousarray(ag[:, None] * wkv[:, :INNER])
    bk = colmaj(ab @ wkv[:, :INNER] + bkv[:INNER], DT)
    wv = np.ascontiguousarray(ag[:, None] * wkv[:, INNER:])
    bv = np.ascontiguousarray(np.broadcast_to(
        ab @ wkv[:, INNER:] + bkv[INNER:], (P, INNER)))
    wo = np.ascontiguousarray(ii["wo"])
    bo = colmaj(ii["bo"], DT)
    # relT rows: head feature d lives at partition (h%2)*64 + d -> duplicate rows
    rT = ii["rel_emb"].T[:, ::-1]  # [64, 1025] column-reversed
    relT = np.ascontiguousarray(np.concatenate([rT, rT], axis=0))  # [128, 1025]

    # c1 columns reordered to match the kernel's half-split loop:
    # half h covers channel blocks ct=4h..4h+3 and lays out [a-cols | g-cols]
    w = ii["conv1_w"]
    HC = CIN // 2  # 512
    c1 = np.ascontiguousarray(np.concatenate(
        [w[:, 0:HC], w[:, CIN:CIN + HC], w[:, HC:CIN], w[:, CIN + HC:]], axis=1))
    c1b = ii["conv1_b"]
    c1a = colmaj(c1b[:CIN], CT)
    c1g = colmaj(c1b[CIN:], CT)
    dwd = np.zeros((CT, P, KW, P), dtype=bf)
    wr = ii["dw_w"].reshape(CT, P, KW).astype(bf)
    pp = np.arange(P)
    for ct in range(CT):
        for k in range(KW):
            dwd[ct, pp, k, pp] = wr[ct, :, k]
    dwd = np.ascontiguousarray(dwd.reshape(CT, P, KW * P))
    inv = 1.0 / np.sqrt(ii["bn_var"] + EPS)
    s = inv * ii["bn_g"]
    t = ii["bn_b"] - ii["bn_mean"] * s
    bns = colmaj(s, CT)
    bnt = colmaj(t + s * ii["dw_b"], CT)
    c2 = np.ascontiguousarray(ii["conv2_w"])
    c2b = colmaj(ii["conv2_b"], DT)

    g3, be3 = ii["ff2_ln_g"], ii["ff2_ln_b"]
    w3 = np.ascontiguousarray(g3[:, None] * ii["ff2_w1"])
    b3 = colmaj(be3 @ ii["ff2_w1"] + ii["ff2_b1"], FT)
    w4 = np.ascontiguousarray((0.5 * ii["ff2_w2"]).astype(bf))
    b4 = colmaj(0.5 * ii["ff2_b2"], DT)

    png = colmaj(ii["pn_g"], DT)
    pnb = colmaj(ii["pn_b"], DT)

    shared = dict(w1=w1, b1=b1, w2=w2, b2=b2, wq=wq, bq=bq, wk=wk, bk=bk,
                  wv=wv, bvb=bv, wo=wo, bo=bo, relT=relT, c1=c1, c1a=c1a,
                  c1g=c1g, dwdiag=dwd, bns=bns, bnt=bnt, c2=c2, c2b=c2b,
                  w3=w3, b3=b3, w4=w4, b4=b4, png=png, pnb=pnb,
                  antid=np.ascontiguousarray(np.eye(P, dtype=f)),
                  onesf=np.ones((P, P), dtype=f))
    x = ii["x"]
    in_maps = []
    for b in range(NCORES):
        m = dict(shared)
        m["xT"] = np.ascontiguousarray(x[b].T)
        in_maps.append(m)
    return in_maps


_BUILT = None


def run(inputs, trace=False):
    global _BUILT
    from concourse import bass_utils

    in_maps = prep_inputs(inputs)
    if _BUILT is None:
        _BUILT = build()
    res = bass_utils.run_bass_kernel_spmd(
        _BUILT, in_maps, core_ids=list(range(NCORES)), trace=trace)
    out = np.stack([np.asarray(r["outT"]).T for r in res.results])
    return np.ascontiguousarray(out.astype(np.float32)), res


def kernel(**inputs):
    out, _ = run(inputs, trace=False)
    return out
